# revision 1
# baseline (speedup 1.0000x reference)
"""Averaged Hausdorff loss kernel for 8 Trainium2 NeuronCores.

Math: for row-sharded blocks, d2[i,j] = |s1_i|^2 + |s2_j|^2 - 2<s1_i, s2_j>
is computed as a single K=5 matmul with augmented operands
    lhsT[:, i] = [x_i, y_i, z_i, n_i, 1]
    rhs[:, j]  = [-2x'_j, -2y'_j, -2z'_j, 1, n'_j]
so each PSUM tile holds squared distances directly.  min_j sqrt(d2) =
sqrt(min_j d2), so only the [128,1] row-mins ever leave the device; the
sqrt + mean (or max) run on host in fp64.

Sharding: core r owns rows [r*1024, (r+1)*1024) of set1 (reduced against
all of set2) and the same rows of set2 (reduced against all of set1).
Both directions are free-dim row-min reductions; no collectives needed.
"""

import sys

sys.path.insert(0, "/opt/trn_rl_repo")

import numpy as np

N_CORES = 8
N = 8192          # set1 rows
M = 8192          # set2 rows
D = 3
ROWS_PER_CORE = N // N_CORES          # 1024
BLOCKS = ROWS_PER_CORE // 128         # 8 row-blocks of 128
CHUNK = 512                           # matmul free dim (one PSUM bank)
GROUP = 4                             # chunks per reduce group (4 banks)
N_GROUPS = M // (CHUNK * GROUP)       # 4 groups of 2048 per block row

_compiled = None


def _build_program():
    import concourse.tile as tile
    from concourse import bacc, mybir

    nc = bacc.Bacc("TRN2", target_bir_lowering=False, debug=False)
    f32 = mybir.dt.float32

    lhs1_d = nc.dram_tensor("lhs1", [5, ROWS_PER_CORE], f32, kind="ExternalInput")
    rhs2_d = nc.dram_tensor("rhs2", [5, M], f32, kind="ExternalInput")
    lhs2_d = nc.dram_tensor("lhs2", [5, ROWS_PER_CORE], f32, kind="ExternalInput")
    rhs1_d = nc.dram_tensor("rhs1", [5, N], f32, kind="ExternalInput")
    out_d = nc.dram_tensor("out", [128, 2 * BLOCKS], f32, kind="ExternalOutput")

    with tile.TileContext(nc) as tc:
        with (
            tc.tile_pool(name="ops", bufs=1) as ops,
            tc.tile_pool(name="psum", bufs=2, space="PSUM") as psum,
            tc.tile_pool(name="small", bufs=1) as small,
        ):
            lhs1 = ops.tile([5, ROWS_PER_CORE], f32, tag="lhs1")
            rhs2 = ops.tile([5, M], f32, tag="rhs2")
            lhs2 = ops.tile([5, ROWS_PER_CORE], f32, tag="lhs2")
            rhs1 = ops.tile([5, N], f32, tag="rhs1")
            nc.sync.dma_start(lhs1[:], lhs1_d[:])
            nc.sync.dma_start(rhs2[:], rhs2_d[:])
            nc.sync.dma_start(lhs2[:], lhs2_d[:])
            nc.sync.dma_start(rhs1[:], rhs1_d[:])

            # group partials: [128, orientation*block, group]
            rowpart = small.tile([128, 2 * BLOCKS, N_GROUPS], f32, tag="rowpart")
            rowmin = small.tile([128, 2 * BLOCKS], f32, tag="rowmin")

            for o, (lhs_s, rhs_s) in enumerate(((lhs1, rhs2), (lhs2, rhs1))):
                for b in range(BLOCKS):
                    ob = o * BLOCKS + b
                    lhsT = lhs_s[:, b * 128 : (b + 1) * 128]
                    for g in range(N_GROUPS):
                        ps = psum.tile([128, GROUP, CHUNK], f32, tag="ps")
                        for c in range(GROUP):
                            j0 = (g * GROUP + c) * CHUNK
                            nc.tensor.matmul(
                                ps[:, c, :], lhsT, rhs_s[:, j0 : j0 + CHUNK]
                            )
                        nc.vector.tensor_reduce(
                            rowpart[:, ob, g : g + 1],
                            ps[:],
                            axis=mybir.AxisListType.XY,
                            op=mybir.AluOpType.min,
                        )

            nc.vector.tensor_reduce(
                rowmin[:],
                rowpart[:],
                axis=mybir.AxisListType.X,
                op=mybir.AluOpType.min,
            )
            nc.sync.dma_start(out_d[:], rowmin[:])

    nc.compile()
    return nc


def _get_program():
    global _compiled
    if _compiled is None:
        _compiled = _build_program()
    return _compiled


def _aug_operands(s):
    """Build [5, n] lhs ([x,y,z,n,1]) and rhs ([-2x,-2y,-2z,1,n]) fp32."""
    s64 = s.astype(np.float64)
    n = (s64 * s64).sum(axis=1)
    ones = np.ones(s.shape[0], dtype=np.float64)
    lhs = np.stack([s64[:, 0], s64[:, 1], s64[:, 2], n, ones]).astype(np.float32)
    rhs = np.stack(
        [-2.0 * s64[:, 0], -2.0 * s64[:, 1], -2.0 * s64[:, 2], ones, n]
    ).astype(np.float32)
    return np.ascontiguousarray(lhs), np.ascontiguousarray(rhs)


def _run_device(s1, s2, trace=False):
    from concourse.bass_utils import run_bass_kernel_spmd

    nc = _get_program()
    lhs1_full, rhs1_full = _aug_operands(s1)
    lhs2_full, rhs2_full = _aug_operands(s2)

    in_maps = []
    for r in range(N_CORES):
        sl = slice(r * ROWS_PER_CORE, (r + 1) * ROWS_PER_CORE)
        in_maps.append(
            {
                "lhs1": np.ascontiguousarray(lhs1_full[:, sl]),
                "rhs2": rhs2_full,
                "lhs2": np.ascontiguousarray(lhs2_full[:, sl]),
                "rhs1": rhs1_full,
            }
        )

    res = run_bass_kernel_spmd(nc, in_maps, list(range(N_CORES)), trace=trace)

    d1min = np.concatenate(
        [res.results[r]["out"][:, 0:BLOCKS].T.reshape(-1) for r in range(N_CORES)]
    )
    d2min = np.concatenate(
        [res.results[r]["out"][:, BLOCKS : 2 * BLOCKS].T.reshape(-1) for r in range(N_CORES)]
    )
    return d1min, d2min, res


def kernel(set1, set2, hausdorff=0, w_set1_set2=1, w_set2_set1=1, n_outputs=1):
    s1 = np.ascontiguousarray(np.asarray(set1, dtype=np.float32))
    s2 = np.ascontiguousarray(np.asarray(set2, dtype=np.float32))
    assert s1.shape == (N, D) and s2.shape == (M, D), (s1.shape, s2.shape)
    hausdorff = int(np.asarray(hausdorff))
    w12 = int(np.asarray(w_set1_set2))
    w21 = int(np.asarray(w_set2_set1))
    n_outputs = int(np.asarray(n_outputs))

    d1min, d2min, _ = _run_device(s1, s2)

    d1 = np.sqrt(np.maximum(d1min, 0.0).astype(np.float64))
    d2 = np.sqrt(np.maximum(d2min, 0.0).astype(np.float64))
    reduce = np.mean if hausdorff == 0 else np.max
    t12 = np.float32(reduce(d1)) if w12 != 0 else np.float32(0.0)
    t21 = np.float32(reduce(d2)) if w21 != 0 else np.float32(0.0)

    if n_outputs == 1:
        return np.float32(t12 + t21)
    return (t12, t21)


# revision 4
# speedup vs baseline: 2.8735x; 2.8735x over previous
"""Averaged Hausdorff loss kernel for 8 Trainium2 NeuronCores.

Math: for row-sharded blocks, d2[i,j] = |s1_i|^2 + |s2_j|^2 - 2<s1_i, s2_j>
is computed as a single K=13 matmul with augmented operands.  Inputs are
split hi/lo into two fp16 parts (x = xh + xl exact to ~2^-22 rel), so the
matmul runs at full PE rate (1 cycle/row vs 4 for fp32) while keeping
~fp32 accuracy: the K rows pair up as
    xh*(-2yh) (3) + xh*(-2yl) (3) + xl*(-2yh) (3) + nh*1 + nl*1 + 1*n'h + 1*n'l
so each PSUM tile holds squared distances directly.  min_j sqrt(d2) =
sqrt(min_j d2), so only the [128,1] row-mins ever leave the device; the
sqrt + mean (or max) run on host in fp64.

Sharding: core r owns rows [r*1024, (r+1)*1024) of set1 (reduced against
all of set2) and the same rows of set2 (reduced against all of set1).
Both directions are free-dim row-min reductions; no collectives needed.
"""

import sys

sys.path.insert(0, "/opt/trn_rl_repo")

import numpy as np

N_CORES = 8
N = 8192          # set1 rows
M = 8192          # set2 rows
D = 3
ROWS_PER_CORE = N // N_CORES          # 1024
BLOCKS = ROWS_PER_CORE // 128         # 8 row-blocks of 128
CHUNK = 512                           # matmul free dim (one PSUM bank)
GROUP = 4                             # chunks per reduce group (4 banks)
N_GROUPS = M // (CHUNK * GROUP)       # 4 groups of 2048 per block row
K = 13                                # augmented contraction dim

_compiled = None


def _build_program():
    import concourse.tile as tile
    from concourse import bacc, mybir

    nc = bacc.Bacc("TRN2", target_bir_lowering=False, debug=False)
    f32 = mybir.dt.float32
    f16 = mybir.dt.float16

    lhs1_d = nc.dram_tensor("lhs1", [K, ROWS_PER_CORE], f16, kind="ExternalInput")
    rhs2_d = nc.dram_tensor("rhs2", [K, M], f16, kind="ExternalInput")
    lhs2_d = nc.dram_tensor("lhs2", [K, ROWS_PER_CORE], f16, kind="ExternalInput")
    rhs1_d = nc.dram_tensor("rhs1", [K, N], f16, kind="ExternalInput")
    out_d = nc.dram_tensor("out", [128, 2 * BLOCKS], f32, kind="ExternalOutput")

    with tile.TileContext(nc) as tc:
        with (
            tc.tile_pool(name="ops", bufs=1) as ops,
            tc.tile_pool(name="psum", bufs=2, space="PSUM") as psum,
            tc.tile_pool(name="small", bufs=1) as small,
        ):
            lhs1 = ops.tile([K, ROWS_PER_CORE], f16, tag="lhs1")
            rhs2 = ops.tile([K, M], f16, tag="rhs2")
            lhs2 = ops.tile([K, ROWS_PER_CORE], f16, tag="lhs2")
            rhs1 = ops.tile([K, N], f16, tag="rhs1")
            nc.sync.dma_start(lhs1[:], lhs1_d[:])
            nc.sync.dma_start(rhs2[:], rhs2_d[:])
            nc.sync.dma_start(lhs2[:], lhs2_d[:])
            nc.sync.dma_start(rhs1[:], rhs1_d[:])

            # group partials: [128, orientation*block, group]
            rowpart = small.tile([128, 2 * BLOCKS, N_GROUPS], f32, tag="rowpart")
            rowmin = small.tile([128, 2 * BLOCKS], f32, tag="rowmin")

            for o, (lhs_s, rhs_s) in enumerate(((lhs1, rhs2), (lhs2, rhs1))):
                for b in range(BLOCKS):
                    ob = o * BLOCKS + b
                    lhsT = lhs_s[:, b * 128 : (b + 1) * 128]
                    for g in range(N_GROUPS):
                        ps = psum.tile([128, GROUP, CHUNK], f32, tag="ps")
                        for c in range(GROUP):
                            j0 = (g * GROUP + c) * CHUNK
                            nc.tensor.matmul(
                                ps[:, c, :], lhsT, rhs_s[:, j0 : j0 + CHUNK]
                            )
                        nc.vector.tensor_reduce(
                            rowpart[:, ob, g : g + 1],
                            ps[:],
                            axis=mybir.AxisListType.XY,
                            op=mybir.AluOpType.min,
                        )

            nc.vector.tensor_reduce(
                rowmin[:],
                rowpart[:],
                axis=mybir.AxisListType.X,
                op=mybir.AluOpType.min,
            )
            nc.sync.dma_start(out_d[:], rowmin[:])

    nc.compile()
    return nc


def _get_program():
    global _compiled
    if _compiled is None:
        _compiled = _build_program()
    return _compiled


def _split16(v):
    """fp64 vector -> (hi, lo) fp16 with v ~= hi + lo to ~2^-22 rel."""
    hi = v.astype(np.float16)
    lo = (v - hi.astype(np.float64)).astype(np.float16)
    return hi.astype(np.float64), lo.astype(np.float64)


def _aug_operands(s):
    """Build [13, n] lhsT and rhs operand stacks in fp16 (hi/lo split)."""
    s64 = s.astype(np.float64)
    n = (s64 * s64).sum(axis=1)
    ones = np.ones(s.shape[0], dtype=np.float64)
    xh = [None] * D
    xl = [None] * D
    for d in range(D):
        xh[d], xl[d] = _split16(s64[:, d])
    nh, nl = _split16(n)
    lhs = np.stack(
        [xh[0], xh[1], xh[2], xh[0], xh[1], xh[2], xl[0], xl[1], xl[2],
         nh, nl, ones, ones]
    ).astype(np.float16)
    rhs = np.stack(
        [-2 * xh[0], -2 * xh[1], -2 * xh[2], -2 * xl[0], -2 * xl[1], -2 * xl[2],
         -2 * xh[0], -2 * xh[1], -2 * xh[2], ones, ones, nh, nl]
    ).astype(np.float16)
    return np.ascontiguousarray(lhs), np.ascontiguousarray(rhs)


def _run_device(s1, s2, trace=False):
    from concourse.bass_utils import run_bass_kernel_spmd

    nc = _get_program()
    lhs1_full, rhs1_full = _aug_operands(s1)
    lhs2_full, rhs2_full = _aug_operands(s2)

    in_maps = []
    for r in range(N_CORES):
        sl = slice(r * ROWS_PER_CORE, (r + 1) * ROWS_PER_CORE)
        in_maps.append(
            {
                "lhs1": np.ascontiguousarray(lhs1_full[:, sl]),
                "rhs2": rhs2_full,
                "lhs2": np.ascontiguousarray(lhs2_full[:, sl]),
                "rhs1": rhs1_full,
            }
        )

    res = run_bass_kernel_spmd(nc, in_maps, list(range(N_CORES)), trace=trace)

    d1min = np.concatenate(
        [res.results[r]["out"][:, 0:BLOCKS].T.reshape(-1) for r in range(N_CORES)]
    )
    d2min = np.concatenate(
        [res.results[r]["out"][:, BLOCKS : 2 * BLOCKS].T.reshape(-1) for r in range(N_CORES)]
    )
    return d1min, d2min, res


def kernel(set1, set2, hausdorff=0, w_set1_set2=1, w_set2_set1=1, n_outputs=1):
    s1 = np.ascontiguousarray(np.asarray(set1, dtype=np.float32))
    s2 = np.ascontiguousarray(np.asarray(set2, dtype=np.float32))
    assert s1.shape == (N, D) and s2.shape == (M, D), (s1.shape, s2.shape)
    hausdorff = int(np.asarray(hausdorff))
    w12 = int(np.asarray(w_set1_set2))
    w21 = int(np.asarray(w_set2_set1))
    n_outputs = int(np.asarray(n_outputs))

    d1min, d2min, _ = _run_device(s1, s2)

    d1 = np.sqrt(np.maximum(d1min, 0.0).astype(np.float64))
    d2 = np.sqrt(np.maximum(d2min, 0.0).astype(np.float64))
    reduce = np.mean if hausdorff == 0 else np.max
    t12 = np.float32(reduce(d1)) if w12 != 0 else np.float32(0.0)
    t21 = np.float32(reduce(d2)) if w21 != 0 else np.float32(0.0)

    if n_outputs == 1:
        return np.float32(t12 + t21)
    return (t12, t21)
